# revision 2
# baseline (speedup 1.0000x reference)
"""Trainium2 Bass kernel for ClassWiseRegressionLoss.

reference semantics:
    idx = labels - 1                       # [N] in [0, C)
    class_pred[i] = pred[i, idx[i], :]     # [N, 2] gather
    d = class_pred - targets               # [N, 2]
    smooth_l1 = where(|d| < 1, 0.5 d^2, |d| - 0.5)
    out = mean(smooth_l1) * 2              # scalar f32

Strategy (data-parallel over N across 8 cores):
  pred is 400 MB but only 2 of 400 floats per row are used. The host
  precomputes flat row indices (n*C + label - 1) per core; each core
  fetches exactly the needed 8-byte pairs from HBM with a handful of
  big SWDGE indirect-DMA gathers whose offset AP covers many proposals
  per partition per instruction (amortizes the ~1us per-instruction
  SWDGE fixed overhead that dominated the per-column-gather variant).
  Smooth-L1 uses the abs-free split
      sum(smooth) = 0.5*(sum(d^2) - sum(r1^2) - sum(m2^2))
  with r1 = max(d-1,0), m2 = min(d+1,0), mapped onto dual-op DVE
  tensor_scalar and ACT Square-with-accumulate ops, chunk-pipelined
  against the gathers. Each core returns [128, 3*nchunk] per-partition
  component sums; the host reduces them in float64 and scales by 1/N.
"""

import functools

import numpy as np

import concourse.bacc as bacc
import concourse.bass as bass
import concourse.mybir as mybir
import concourse.tile as tile
from concourse.bass import IndirectOffsetOnAxis
from concourse.bass_utils import run_bass_kernel_spmd

N = 262144
C = 200
NCORES = 8
NLOC = N // NCORES  # 32768 proposals per core
P = 128  # SBUF partitions
K = NLOC // P  # 256 proposals per partition
NCHUNK = 2  # number of indirect-gather instructions (pipeline depth)

f32 = mybir.dt.float32
i32 = mybir.dt.int32


@functools.lru_cache(maxsize=2)
def _build(nchunk: int = NCHUNK):
    kc = K // nchunk  # proposals per partition per gather
    nc = bacc.Bacc(None, target_bir_lowering=False, debug=False)

    pred_t = nc.declare_dram_parameter("pred", [NLOC * C, 2], f32, isOutput=False)
    idx_t = nc.declare_dram_parameter("idx", [NLOC], i32, isOutput=False)
    tgt_t = nc.declare_dram_parameter("targets", [NLOC, 2], f32, isOutput=False)
    # per-partition component sums per chunk:
    # col 3c+0 = sum(d^2), 3c+1 = sum(r1^2), 3c+2 = sum(m2^2)
    out_t = nc.declare_dram_parameter("partial", [P, 3 * nchunk], f32, isOutput=True)

    with tile.TileContext(nc) as tc:
        with (
            tc.tile_pool(name="io", bufs=1) as io,
            tc.tile_pool(name="work", bufs=2) as work,
        ):
            idx = io.tile([P, K], i32)
            nc.sync.dma_start(out=idx[:], in_=idx_t[:].rearrange("(p k) -> p k", p=P))
            tg = io.tile([P, 2 * K], f32)
            nc.sync.dma_start(
                out=tg[:], in_=tgt_t[:].rearrange("(p k) two -> p (k two)", p=P)
            )

            g_all = io.tile([P, 2 * K], f32)
            part = io.tile([P, 3 * nchunk], f32)
            for ci in range(nchunk):
                ks = slice(ci * kc, (ci + 1) * kc)
                es = slice(2 * ci * kc, 2 * (ci + 1) * kc)
                # one instruction gathers kc 8-byte pairs per partition:
                # offset (p, k) -> pred row idx[p, k] -> g_all[p, 2k:2k+2]
                nc.gpsimd.indirect_dma_start(
                    out=g_all[:, es],
                    out_offset=None,
                    in_=pred_t[:, :],
                    in_offset=IndirectOffsetOnAxis(ap=idx[:, ks], axis=0),
                )

                d = work.tile([P, 2 * kc], f32)
                nc.vector.tensor_tensor(
                    out=d[:], in0=g_all[:, es], in1=tg[:, es],
                    op=mybir.AluOpType.subtract,
                )
                # relu(|d|-1)^2 = r1^2 + m2^2, r1 = max(d-1,0), m2 = min(d+1,0)
                r1 = work.tile([P, 2 * kc], f32)
                nc.vector.tensor_scalar(
                    out=r1[:], in0=d[:], scalar1=-1.0, scalar2=0.0,
                    op0=mybir.AluOpType.add, op1=mybir.AluOpType.max,
                )
                m2 = work.tile([P, 2 * kc], f32)
                nc.vector.tensor_scalar(
                    out=m2[:], in0=d[:], scalar1=1.0, scalar2=0.0,
                    op0=mybir.AluOpType.add, op1=mybir.AluOpType.min,
                )
                d2 = work.tile([P, 2 * kc], f32)
                nc.scalar.activation(
                    out=d2[:], in_=d[:],
                    func=mybir.ActivationFunctionType.Square,
                    accum_out=part[:, 3 * ci : 3 * ci + 1],
                )
                r12 = work.tile([P, 2 * kc], f32)
                nc.scalar.activation(
                    out=r12[:], in_=r1[:],
                    func=mybir.ActivationFunctionType.Square,
                    accum_out=part[:, 3 * ci + 1 : 3 * ci + 2],
                )
                m22 = work.tile([P, 2 * kc], f32)
                nc.scalar.activation(
                    out=m22[:], in_=m2[:],
                    func=mybir.ActivationFunctionType.Square,
                    accum_out=part[:, 3 * ci + 2 : 3 * ci + 3],
                )
            nc.sync.dma_start(out=out_t[:, :], in_=part[:])

    nc.compile()
    return nc


@functools.lru_cache(maxsize=1)
def _row_base() -> np.ndarray:
    # local row base n*C - 1 so that idx = base + label gives n*C + (label-1)
    return np.arange(NLOC, dtype=np.int64) * C - 1


def _run(pred, labels, targets, trace=False, nchunk: int = NCHUNK):
    pred = np.asarray(pred, dtype=np.float32)
    labels = np.asarray(labels)
    targets = np.asarray(targets, dtype=np.float32)
    assert pred.shape == (N, C, 2), pred.shape
    assert labels.shape == (N,), labels.shape
    assert targets.shape == (N, 2), targets.shape

    base = _row_base()

    nc = _build(nchunk)
    in_maps = []
    for c in range(NCORES):
        sl = slice(c * NLOC, (c + 1) * NLOC)
        idx = (labels[sl].astype(np.int64) + base).astype(np.int32)
        in_maps.append(
            {
                "pred": pred[sl].reshape(NLOC * C, 2),
                "idx": idx,
                "targets": targets[sl],
            }
        )
    res = run_bass_kernel_spmd(nc, in_maps, list(range(NCORES)), trace=trace)
    total = 0.0
    for r in res.results:
        p = r["partial"].astype(np.float64).reshape(P, -1, 3)
        # sum(smooth_l1) = 0.5*(sum(d^2) - sum(r1^2) - sum(m2^2))
        total += 0.5 * (p[:, :, 0].sum() - p[:, :, 1].sum() - p[:, :, 2].sum())
    loss = np.float32(total / N)  # = mean * 2 over 2N elements
    return loss, res


def kernel(pred, labels, targets):
    loss, _ = _run(pred, labels, targets)
    return np.asarray(loss, dtype=np.float32)
